# revision 18
# baseline (speedup 1.0000x reference)
"""Trainium2 Bass kernel for the multi-omics GNN encoder (nn_Encoder_overall).

Sharding: node dimension N=4096 is split across 8 cores (512 rows each).
Params replicated. Three AllGathers move K/V/x projections, e2 + hypergraph
partials, and the decoder input Y between cores.

Attention is computed in transposed orientation sT[j,i] = K Q^T so that the
PV product contracts over the partition axis naturally.  Masks are streamed
row-major (contiguous DMA), converted int32->bf16 on GpSimd, transposed on
the PE, and applied multiplicatively to exp(s).  Row sums ride along as a
ones-column in the PV stationary operand.

f32 on Q/K/score paths; bf16 for p/V/masks/adj/Y (validated ~7e-4 rel err).
"""

import sys

sys.path.insert(0, "/opt/trn_rl_repo")

import numpy as np

import concourse.bass as bass
import concourse.tile as tile
from concourse import bacc, mybir
from concourse import bass_utils
from concourse.masks import make_identity

F32 = mybir.dt.float32
BF16 = mybir.dt.bfloat16
I32 = mybir.dt.int32
AF = mybir.ActivationFunctionType
ALU = mybir.AluOpType
AX = mybir.AxisListType

DEBUG = False

NCORES = 8
N = 4096
S = N // NCORES          # 512 local rows
IT = S // 128            # 4 i-tiles
JT = N // 128            # 32 j-tiles
JC = 8                   # j-chunks of 512
D = 64
HD = 32
DIN1 = 1024
DIN2 = 256
NE = 512
RG = [list(range(NCORES))]

# ---- AG-A dram layout (rows of 1024 bf16 per rank block) ----
# V1:0-31  V2:32-63  x1:64-95  x2:96-127  K12T:128-255(f32)  de:256  (pad->258)
AGA_ROWS = 258

ENC_SPECS = {
    "enc1": dict(din=DIN1, kt=DIN1 // 128),
    "enc2": dict(din=DIN2, kt=DIN2 // 128),
}

PARAM_SHAPES = {}
for _e, _d in (("enc1", DIN1), ("enc2", DIN2)):
    PARAM_SHAPES.update({
        f"{_e}_qw": (_d, D), f"{_e}_qb": (D,), f"{_e}_kw": (_d, D), f"{_e}_kb": (D,),
        f"{_e}_vw": (_d, D), f"{_e}_vb": (D,), f"{_e}_ow": (D, D), f"{_e}_ob": (D,),
        f"{_e}_rw": (_d, D), f"{_e}_rb": (D,), f"{_e}_ln_g": (D,), f"{_e}_ln_b": (D,),
        f"{_e}_f1w": (D, D), f"{_e}_f1b": (D,), f"{_e}_f2w": (D, D), f"{_e}_f2b": (D,),
    })
for _s, _d in (("sp1", DIN1), ("sp2", DIN2)):
    PARAM_SHAPES.update({
        f"{_s}_fw": (_d, D), f"{_s}_fb": (D,), f"{_s}_cw": (2, D), f"{_s}_cb": (D,),
        f"{_s}_gw": (D, D), f"{_s}_ln_g": (D,), f"{_s}_ln_b": (D,),
    })
PARAM_SHAPES.update({
    "dec1": (D, DIN1), "dec2": (D, DIN2),
    "att1_w": (D, D), "att1_u": (D, 1), "att2_w": (D, D), "att2_u": (D, 1),
    "hg": (2 * D, D),
    "cross_qw": (D, D), "cross_kw": (D, D), "cross_vw": (D, D),
    "caus_qw": (D, D), "caus_qb": (D,), "caus_kw": (D, D), "caus_kb": (D,),
    "caus_vw": (D, D), "caus_vb": (D,),
})


def _flatten_params(params):
    out = {}
    for e in ("enc1", "enc2"):
        for k, v in params[e].items():
            out[f"{e}_{k}"] = np.asarray(v)
    for s in ("sp1", "sp2"):
        for k, v in params[s].items():
            out[f"{s}_{k}"] = np.asarray(v)
    out["dec1"] = np.asarray(params["dec1"])
    out["dec2"] = np.asarray(params["dec2"])
    for a in ("att1", "att2"):
        out[f"{a}_w"] = np.asarray(params[a]["w"])
        out[f"{a}_u"] = np.asarray(params[a]["u"])
    out["hg"] = np.asarray(params["hg"])
    for k, v in params["cross"].items():
        out[f"cross_{k}"] = np.asarray(v)
    for k, v in params["caus"].items():
        out[f"caus_{k}"] = np.asarray(v)
    return out


def build_kernel():
    nc = bacc.Bacc("TRN2", target_bir_lowering=False, debug=False,
                   enable_asserts=False, num_devices=NCORES)

    # ---------------- DRAM I/O ----------------
    f1d = nc.dram_tensor("f1", [S, DIN1], F32, kind="ExternalInput")
    f2d = nc.dram_tensor("f2", [S, DIN2], F32, kind="ExternalInput")
    coordsd = nc.dram_tensor("coords", [S, 2], F32, kind="ExternalInput")
    adjd = nc.dram_tensor("adj", [S, N], F32, kind="ExternalInput")
    Hd = nc.dram_tensor("H", [S, NE], F32, kind="ExternalInput")
    m1d = nc.dram_tensor("m1", [S, N], I32, kind="ExternalInput")
    m2d = nc.dram_tensor("m2", [S, N], I32, kind="ExternalInput")
    cmd = nc.dram_tensor("cm", [S, N], I32, kind="ExternalInput")
    pd = {k: nc.dram_tensor(f"p_{k}", list(v), F32, kind="ExternalInput")
          for k, v in PARAM_SHAPES.items()}
    outd = nc.dram_tensor("out", [S, DIN1 + DIN2], F32, kind="ExternalOutput")
    dbg = {}
    if DEBUG:
        for nm in ("ef1", "ef2", "es1", "es2", "e1", "e2", "att", "caus", "hyp",
                   "fin1", "fin2"):
            dbg[nm] = nc.dram_tensor(f"dbg_{nm}", [S, D], F32, kind="ExternalOutput")
        dbg["qt"] = nc.dram_tensor("dbg_qt", [128, S], F32, kind="ExternalOutput")
        dbg["kt0"] = nc.dram_tensor("dbg_kt0", [128, S], F32, kind="ExternalOutput")
        dbg["vpv"] = nc.dram_tensor("dbg_vpv", [128, 4 * 66], BF16, kind="ExternalOutput")
        dbg["msl"] = nc.dram_tensor("dbg_msl", [128, 1024], BF16, kind="ExternalOutput")
        dbg["pm0"] = nc.dram_tensor("dbg_pm0", [128, 1024], BF16, kind="ExternalOutput")
        dbg["h12"] = nc.dram_tensor("dbg_h12", [64, S], F32, kind="ExternalOutput")
        dbg["rs"] = nc.dram_tensor("dbg_rs", [64, S], F32, kind="ExternalOutput")
        dbg["hn"] = nc.dram_tensor("dbg_hn", [64, S], F32, kind="ExternalOutput")
        dbg["ln1"] = nc.dram_tensor("dbg_ln1", [S, D], F32, kind="ExternalOutput")

    with tile.TileContext(nc) as tc:
        _body(tc, f1d, f2d, coordsd, adjd, Hd, m1d, m2d, cmd, pd, outd, dbg)

    nc.compile()
    return nc


def _body(tc, f1d, f2d, coordsd, adjd, Hd, m1d, m2d, cmd, pd, outd, dbg):
    nc = tc.nc
    from contextlib import ExitStack
    es_ctx = ExitStack()
    glob = es_ctx.enter_context(tc.tile_pool(name="glob", bufs=1))
    dram = es_ctx.enter_context(tc.tile_pool(name="dram", bufs=1, space="DRAM"))

    def gt(shape, dtype, name):
        return glob.tile(shape, dtype, name=name)

    # ---------------- constants ----------------
    idf = gt([128, 128], F32, "idf")
    make_identity(nc, idf)
    idb = gt([128, 128], BF16, "idb")
    make_identity(nc, idb)
    ones_col = gt([128, 1], F32, "ones_col")
    nc.vector.memset(ones_col, 1.0)
    epsln = gt([128, 1], F32, "epsln")
    nc.vector.memset(epsln, 1e-5)

    def ap_bcast(dram_h, p, f):
        return bass.AP(tensor=dram_h, offset=0, ap=[[0, p], [1, f]])

    def bcast64(key):
        t = gt([128, D], F32, f"bc_{key}")
        nc.sync.dma_start(out=t, in_=ap_bcast(pd[key], 128, D))
        return t

    def col64(key, rows=D):
        t = gt([rows, 1], F32, f"cv_{key}")
        nc.sync.dma_start(out=t, in_=pd[key].ap().rearrange("(d one) -> d one", one=1))
        return t

    def mat(key, shape=None):
        shape = shape or list(PARAM_SHAPES[key])
        t = gt(list(shape), F32, f"m_{key}")
        nc.sync.dma_start(out=t, in_=pd[key].ap())
        return t

    bc = {k: bcast64(k) for k in (
        "enc1_rb", "enc1_ob", "enc1_f1b", "enc1_f2b", "enc1_ln_g", "enc1_ln_b",
        "enc2_rb", "enc2_ob", "enc2_f1b", "enc2_f2b", "enc2_ln_g", "enc2_ln_b",
        "sp1_fb", "sp1_cb", "sp1_ln_g", "sp1_ln_b",
        "sp2_fb", "sp2_cb", "sp2_ln_g", "sp2_ln_b",
    )}
    vbT = {e: col64(f"{e}_vb") for e in ("enc1", "enc2")}
    vbT["caus"] = col64("caus_vb")
    qbcT = col64("caus_qb")
    kbcT = col64("caus_kb")

    qkb = {}
    for e in ("enc1", "enc2"):
        t = gt([128, 1], F32, f"qkb_{e}")
        nc.sync.dma_start(out=t[0:64, :], in_=pd[f"{e}_qb"].ap().rearrange("(d one) -> d one", one=1))
        nc.sync.dma_start(out=t[64:128, :], in_=pd[f"{e}_kb"].ap().rearrange("(d one) -> d one", one=1))
        qkb[e] = t

    ow = {e: mat(f"{e}_ow") for e in ("enc1", "enc2")}
    f1w = {e: mat(f"{e}_f1w") for e in ("enc1", "enc2")}
    f2w = {e: mat(f"{e}_f2w") for e in ("enc1", "enc2")}
    gw = {s: mat(f"{s}_gw") for s in ("sp1", "sp2")}
    cwm = {s: mat(f"{s}_cw") for s in ("sp1", "sp2")}
    attw = {a: mat(f"{a}_w") for a in ("att1", "att2")}
    attu = {a: mat(f"{a}_u") for a in ("att1", "att2")}
    hgsb = mat("hg")
    dec1sb = mat("dec1")
    dec2sb = mat("dec2")
    crossw = {k: mat(f"cross_{k}") for k in ("qw", "kw", "vw")}
    causw = {k: mat(f"caus_{k}") for k in ("qw", "kw", "vw")}

    # ---------------- helpers ----------------
    scr = es_ctx.enter_context(tc.tile_pool(name="scratch", bufs=3))

    def layernorm(out_ap, x_ap, g_bc, b_bc):
        """out = LN(x) * g + b, per 128-row tile, free dim D."""
        stats = scr.tile([128, 6], F32, name="ln_stats", tag="ln_stats")
        nc.vector.bn_stats(out=stats, in_=x_ap)
        mv = scr.tile([128, 2], F32, name="ln_mv", tag="ln_mv")
        nc.vector.bn_aggr(out=mv, in_=stats)
        lnv = scr.tile([128, 1], F32, name="ln_lnv", tag="ln_lnv")
        nc.scalar.activation(lnv, mv[:, 1:2], AF.Ln, bias=epsln)
        rstd = scr.tile([128, 1], F32, name="ln_rstd", tag="ln_rstd")
        nc.scalar.activation(rstd, lnv, AF.Exp, scale=-0.5)
        xm = scr.tile([128, D], F32, name="ln_xm", tag="ln_xm")
        nc.vector.tensor_scalar(out=xm, in0=x_ap, scalar1=mv[:, 0:1], scalar2=None,
                                op0=ALU.subtract)
        nc.vector.tensor_scalar(out=xm, in0=xm, scalar1=rstd, scalar2=None, op0=ALU.mult)
        nc.vector.tensor_tensor(out=xm, in0=xm, in1=g_bc, op=ALU.mult)
        nc.vector.tensor_tensor(out=out_ap, in0=xm, in1=b_bc, op=ALU.add)

    # cross-phase carriers (small)
    r_sb = {e: gt([128, IT, D], F32, f"rsb_{e}") for e in ("enc1", "enc2")}
    ef = {e: gt([128, IT, D], F32, f"ef_{e}") for e in ("enc1", "enc2")}
    es_m = {s: gt([128, IT, D], F32, f"es_{s}") for s in ("sp1", "sp2")}
    adjT = gt([128, JT, S], BF16, "adjT")
    dvis = gt([128, IT, 1], F32, "dvis")
    deinvT = gt([128, IT, 1], F32, "deinvT")
    att_nat = gt([128, IT, D], F32, "att_nat")
    caus_nat = gt([128, IT, D], F32, "caus_nat")
    hyp = gt([128, IT, D], F32, "hyp")
    fin1 = gt([128, IT, D], F32, "fin1")
    fin2 = gt([128, IT, D], F32, "fin2")

    hctx = ExitStack()
    hpool = hctx.enter_context(tc.tile_pool(name="hpool", bufs=1))
    H_sb = hpool.tile([128, IT, NE], F32, name="H_sb")
    nc.sync.dma_start(out=H_sb, in_=Hd.ap().rearrange("(it p) ne -> p it ne", it=IT))

    # =========================================================
    # PHASE 1: local loads, transposes, projections
    # =========================================================
    enc_ctx = ExitStack()
    encdat = enc_ctx.enter_context(tc.tile_pool(name="encdat", bufs=1))
    ph1 = ExitStack()
    p1io = ph1.enter_context(tc.tile_pool(name="p1io", bufs=1))
    ps1 = ph1.enter_context(tc.tile_pool(name="ps1", bufs=2, space="PSUM"))
    ps1t = ph1.enter_context(tc.tile_pool(name="ps1t", bufs=2, space="PSUM"))

    f1sb = p1io.tile([128, IT, DIN1], F32, name="f1sb")
    nc.sync.dma_start(out=f1sb, in_=f1d.ap().rearrange("(it p) k -> p it k", it=IT))
    f2sb = p1io.tile([128, IT, DIN2], F32, name="f2sb")
    nc.sync.dma_start(out=f2sb, in_=f2d.ap().rearrange("(it p) k -> p it k", it=IT))
    coords_sb = p1io.tile([128, IT, 2], F32, name="coords_sb")
    nc.sync.dma_start(out=coords_sb, in_=coordsd.ap().rearrange("(it p) k -> p it k", it=IT))

    # f1T / f2T
    fT = {}
    for e, fsb, din in (("enc1", f1sb, DIN1), ("enc2", f2sb, DIN2)):
        kt = din // 128
        t = p1io.tile([128, kt, S], F32, name=f"fT_{e}")
        for k in range(kt):
            ps = ps1t.tile([128, S], F32, name="fT_ps", tag="fT_ps")
            for it in range(IT):
                nc.tensor.transpose(ps[:, it * 128:(it + 1) * 128],
                                    fsb[:, it, k * 128:(k + 1) * 128], idf)
            nc.vector.tensor_copy(out=t[:, k, :], in_=ps)
        fT[e] = t

    coordsT = p1io.tile([2, S], F32, name="coordsT")
    for it in range(IT):
        ps = ps1t.tile([128, 128], F32, name="cT_ps", tag="fT_ps")
        nc.tensor.transpose(ps[0:2, :], coords_sb[:, it, :], idf)
        nc.vector.tensor_copy(out=coordsT[:, it * 128:(it + 1) * 128], in_=ps[0:2, :])

    # QT stack / K-local stack
    QTstack = encdat.tile([128, S], F32, name="QTstack")
    kstack_local = p1io.tile([128, S], F32, name="kstack_local")
    for gi, e in enumerate(("enc1", "enc2")):
        kt = ENC_SPECS[e]["kt"]
        qkw_t = p1io.tile([128, kt, 128], F32, name=f"qkw_{e}")
        nc.sync.dma_start(out=qkw_t[:, :, 0:64], in_=pd[f"{e}_qw"].ap().rearrange("(kt p) d -> p kt d", kt=kt))
        nc.sync.dma_start(out=qkw_t[:, :, 64:128], in_=pd[f"{e}_kw"].ap().rearrange("(kt p) d -> p kt d", kt=kt))
        ps = ps1.tile([128, S], F32, name="qk_ps", tag="qk_ps")
        for k in range(kt):
            nc.tensor.matmul(ps, qkw_t[:, k, :], fT[e][:, k, :],
                             start=(k == 0), stop=(k == kt - 1))
        nc.scalar.activation(QTstack[gi * 64:gi * 64 + 64, :], ps[0:64, :],
                             AF.Identity, bias=qkb[e][0:64, :])
        nc.scalar.activation(kstack_local[gi * 64:gi * 64 + 64, :], ps[64:128, :],
                             AF.Identity, bias=qkb[e][64:128, :])

    # V/R/feat and spatial x per modality
    v_loc = {}
    x_loc_bf = {}
    for e, sp in (("enc1", "sp1"), ("enc2", "sp2")):
        kt = ENC_SPECS[e]["kt"]
        vrf_t = p1io.tile([128, kt, 192], F32, name=f"vrf_{e}")
        nc.sync.dma_start(out=vrf_t[:, :, 0:64], in_=pd[f"{e}_vw"].ap().rearrange("(kt p) d -> p kt d", kt=kt))
        nc.sync.dma_start(out=vrf_t[:, :, 64:128], in_=pd[f"{e}_rw"].ap().rearrange("(kt p) d -> p kt d", kt=kt))
        nc.sync.dma_start(out=vrf_t[:, :, 128:192], in_=pd[f"{sp}_fw"].ap().rearrange("(kt p) d -> p kt d", kt=kt))
        vt = p1io.tile([128, IT, D], BF16, name=f"vloc_{e}")
        xbf = p1io.tile([128, IT, D], BF16, name=f"xbf_{e}")
        for it in range(IT):
            ps = ps1.tile([128, 192], F32, name="vrf_ps", tag="vrf_ps")
            for k in range(kt):
                nc.tensor.matmul(ps, fT[e][:, k, it * 128:(it + 1) * 128],
                                 vrf_t[:, k, :], start=(k == 0), stop=(k == kt - 1))
            nc.vector.tensor_copy(out=vt[:, it, :], in_=ps[:, 0:64])
            nc.vector.tensor_tensor(out=r_sb[e][:, it, :], in0=ps[:, 64:128],
                                    in1=bc[f"{e}_rb"], op=ALU.add)
            fa = scr.tile([128, D], F32, name="feat_a", tag="feat_a")
            nc.vector.tensor_tensor(out=fa, in0=ps[:, 128:192], in1=bc[f"{sp}_fb"], op=ALU.add)
            nc.scalar.activation(fa, fa, AF.Relu)
            cps = ps1.tile([128, D], F32, name="featc_ps", tag="featc_ps", bufs=1)
            nc.tensor.matmul(cps, coordsT[:, it * 128:(it + 1) * 128], cwm[sp])
            fb_ = scr.tile([128, D], F32, name="feat_b", tag="feat_b")
            nc.vector.tensor_tensor(out=fb_, in0=cps, in1=bc[f"{sp}_cb"], op=ALU.add)
            nc.scalar.activation(fb_, fb_, AF.Relu)
            nc.vector.tensor_tensor(out=fa, in0=fa, in1=fb_, op=ALU.add)
            xl = scr.tile([128, D], F32, name="x_ln", tag="x_ln")
            layernorm(xl, fa, bc[f"{sp}_ln_g"], bc[f"{sp}_ln_b"])
            nc.vector.tensor_copy(out=xbf[:, it, :], in_=xl)
        v_loc[e] = vt
        x_loc_bf[e] = xbf

    # de partial: column sums of H  [1, NE]
    de_ps = ps1.tile([1, NE], F32, name="de_ps", tag="de_ps", bufs=1)
    for it in range(IT):
        nc.tensor.matmul(de_ps, ones_col, H_sb[:, it, :],
                         start=(it == 0), stop=(it == IT - 1))
    de_part = p1io.tile([1, NE], F32, name="de_part")
    nc.vector.tensor_copy(out=de_part, in_=de_ps)

    # ---------------- AG-A ----------------
    agA_in = dram.tile([AGA_ROWS, 1024], BF16, name="agA_in")
    agA_out = dram.tile([AGA_ROWS * NCORES, 1024], BF16, name="agA_out",
                        addr_space="Shared")

    for r0, t in ((0, v_loc["enc1"]), (32, v_loc["enc2"]),
                  (64, x_loc_bf["enc1"]), (96, x_loc_bf["enc2"])):
        nc.sync.dma_start(
            out=agA_in[r0:r0 + 32, :].rearrange("(it ph) (pl d) -> (ph pl) it d",
                                                it=IT, ph=8, pl=16, d=D),
            in_=t)
    nc.sync.dma_start(out=agA_in[128:256, :].bitcast(F32), in_=kstack_local)
    nc.sync.dma_start(out=agA_in[256:257, :].bitcast(F32), in_=de_part)

    nc.gpsimd.collective_compute(
        "AllGather", ALU.bypass, replica_groups=RG,
        ins=[agA_in.opt()], outs=[agA_out.opt()])

    ph1.close()

    # ---------------- AG-A read-back ----------------
    Vpv = {}
    for e, r0 in (("enc1", 0), ("enc2", 32)):
        t = encdat.tile([128, JT, 66], BF16, name=f"Vpv_{e}")
        nc.vector.memset(t.rearrange("p jt (h q) -> p jt h q", h=2, q=33)[:, :, :, 32:33], 1.0)
        for r in range(NCORES):
            src = agA_out[AGA_ROWS * r + r0: AGA_ROWS * r + r0 + 32, :]
            srcv = src.rearrange("(it ph) (pl h dl) -> (ph pl) it h dl",
                                 it=IT, ph=8, pl=16, h=2, dl=32)
            for h in range(2):
                nc.sync.dma_start(
                    out=t[:, 4 * r:4 * r + 4, 33 * h:33 * h + 32],
                    in_=srcv[:, :, h, :])
        Vpv[e] = t

    xfull = encdat.tile([128, JT, 128], BF16, name="xfull")
    for r in range(NCORES):
        for col0, r0 in ((0, 64), (64, 96)):
            src = agA_out[AGA_ROWS * r + r0: AGA_ROWS * r + r0 + 32, :]
            nc.sync.dma_start(
                out=xfull[:, 4 * r:4 * r + 4, col0:col0 + 64],
                in_=src.rearrange("(it ph) (pl d) -> (ph pl) it d", it=IT, ph=8, pl=16, d=D))

    K12T = encdat.tile([128, NCORES, S], F32, name="K12T")
    for r in range(NCORES):
        nc.sync.dma_start(out=K12T[:, r, :],
                          in_=agA_out[AGA_ROWS * r + 128: AGA_ROWS * r + 256, :].bitcast(F32))

    # de -> de_inv -> deinvT (natural) ; done early, freed early
    with tc.tile_pool(name="dep", bufs=1) as depool:
        de8 = depool.tile([1, NCORES, NE], F32, name="de8")
        for r in range(NCORES):
            nc.sync.dma_start(out=de8[:, r, :],
                              in_=agA_out[AGA_ROWS * r + 256: AGA_ROWS * r + 257, :].bitcast(F32))
        nc.vector.tensor_tensor(out=de8[:, 0:4, :], in0=de8[:, 0:4, :],
                                in1=de8[:, 4:8, :], op=ALU.add)
        nc.vector.tensor_tensor(out=de8[:, 0:2, :], in0=de8[:, 0:2, :],
                                in1=de8[:, 2:4, :], op=ALU.add)
        de_f = depool.tile([1, NE], F32, name="de_f")
        nc.vector.tensor_tensor(out=de_f, in0=de8[:, 0, :], in1=de8[:, 1, :], op=ALU.add)
        nc.vector.tensor_scalar(out=de_f, in0=de_f, scalar1=1e-6, scalar2=None, op0=ALU.add)
        nc.vector.reciprocal(out=de_f, in_=de_f)
        deinv_d = dram.tile([1, NE], F32, name="deinv_d")
        nc.sync.dma_start(out=deinv_d, in_=de_f)
        nc.sync.dma_start(out=deinvT,
                          in_=deinv_d.rearrange("x (net p) -> p net x", net=IT))

    if dbg:
        nc.sync.dma_start(out=dbg["qt"].ap(), in_=QTstack)
        nc.sync.dma_start(out=dbg["kt0"].ap(), in_=K12T[:, 0, :])
        nc.sync.dma_start(out=dbg["vpv"].ap(),
                          in_=Vpv["enc1"][:, 0:4, :].rearrange("p a b -> p (a b)"))

    # =========================================================
    # attention pass helper
    # =========================================================
    def attention_pass(tag, groups, mask_dram, scale, qt, ktiles):
        ctx = ExitStack()
        p_sT = ctx.enter_context(tc.tile_pool(name=f"sT_{tag}", bufs=2, space="PSUM"))
        p_hT = ctx.enter_context(tc.tile_pool(name=f"hT_{tag}", bufs=1, space="PSUM"))
        p_tp = ctx.enter_context(tc.tile_pool(name=f"tp_{tag}", bufs=2, space="PSUM"))
        p_ms = ctx.enter_context(tc.tile_pool(name=f"ms_{tag}", bufs=2))
        p_pm = ctx.enter_context(tc.tile_pool(name=f"pm_{tag}", bufs=3))

        hT = [p_hT.tile([g["hM"], S], F32, name=f"hT{gi}", tag=f"hT{gi}")
              for gi, g in enumerate(groups)]

        any_masked = any(g["masked"] for g in groups)
        for jc in range(JC):
            mslab = None
            if any_masked and mask_dram is not None:
                mint = p_ms.tile([128, IT, 512], I32, name="mint", tag="mint", bufs=1)
                src = mask_dram.ap().rearrange("(it p) (jc w) -> p it jc w",
                                               it=IT, jc=JC)[:, :, jc, :]
                nc.sync.dma_start(out=mint, in_=src)
                mbf = p_ms.tile([128, IT, 512], BF16, name="mbf", tag="mbf")
                nc.gpsimd.tensor_copy(out=mbf, in_=mint)
                mslab = [p_ms.tile([128, 1024], BF16, name=f"mT{h}", tag=f"mT{h}")
                         for h in range(2)]
                for jtw in range(4):
                    tp = p_tp.tile([128, 512], BF16, name="mtp", tag="mtp")
                    for it in range(IT):
                        nc.tensor.transpose(tp[:, it * 128:(it + 1) * 128],
                                            mbf[:, it, jtw * 128:(jtw + 1) * 128], idb)
                    nc.vector.tensor_copy(
                        out=mslab[jtw // 2][:, (jtw % 2) * 512:(jtw % 2) * 512 + 512],
                        in_=tp)
            if dbg and tag == "enc1" and jc == 0 and mslab is not None:
                nc.sync.dma_start(out=dbg["msl"].ap(), in_=mslab[0])
            for gi, g in enumerate(groups):
                rb, K = g["row_base"], g["K"]
                for half in range(2):
                    sT = p_sT.tile([128, 1024], F32, name="sT", tag="sT")
                    for sub in range(2):
                        jtw = half * 2 + sub
                        nc.tensor.matmul(
                            sT[:, sub * 512:sub * 512 + 512],
                            ktiles[rb:rb + K, jc, jtw * 128:(jtw + 1) * 128],
                            qt[rb:rb + K, :],
                            start=True, stop=True, tile_position=(rb, 0))
                    p = p_pm.tile([128, 1024], BF16, name="pexp", tag="pexp")
                    nc.scalar.activation(p, sT, AF.Exp, scale=scale)
                    if g["masked"]:
                        pm = p_pm.tile([128, 1024], BF16, name="pmask", tag="pmask")
                        nc.vector.tensor_tensor(out=pm, in0=p, in1=mslab[half], op=ALU.mult)
                    else:
                        pm = p
                    if dbg and tag == "enc1" and jc == 0 and gi == 0 and half == 0:
                        nc.sync.dma_start(out=dbg["pm0"].ap(), in_=pm)
                    for sub in range(2):
                        jtw = half * 2 + sub
                        jt = jc * 4 + jtw
                        nc.tensor.matmul(hT[gi], g["vpv"](jt),
                                         pm[:, sub * 512:sub * 512 + 512],
                                         start=(jt == 0), stop=(jt == JT - 1))
        return hT, ctx

    # =========================================================
    # PHASE 2: enc1 / enc2 graph transformers
    # =========================================================
    for gi, e in enumerate(("enc1", "enc2")):
        vt = Vpv[e]
        groups = [
            dict(row_base=gi * 64, K=HD, masked=True, hM=33,
                 vpv=lambda jt, vt=vt: vt[:, jt, 0:33]),
            dict(row_base=gi * 64 + 32, K=HD, masked=True, hM=33,
                 vpv=lambda jt, vt=vt: vt[:, jt, 33:66]),
        ]
        mask_d = m1d if e == "enc1" else m2d
        epc = ExitStack()
        epl = epc.enter_context(tc.tile_pool(name=f"epl_{e}", bufs=1))
        hT, actx = attention_pass(e, groups, mask_d, 1.0 / float(np.sqrt(HD)),
                                  QTstack, K12T)

        h12 = epl.tile([64, S], F32, name=f"h12_{e}")
        rs_a = epl.tile([1, S], F32, name=f"rs_a_{e}")
        rs_b = epl.tile([1, S], F32, name=f"rs_b_{e}")
        nc.scalar.copy(h12[0:32, :], hT[0][0:32, :])
        nc.scalar.copy(h12[32:64, :], hT[1][0:32, :])
        nc.scalar.copy(rs_a, hT[0][32:33, :])
        nc.scalar.copy(rs_b, hT[1][32:33, :])
        actx.close()
        nc.vector.reciprocal(out=rs_a, in_=rs_a)
        nc.vector.reciprocal(out=rs_b, in_=rs_b)
        rsd = dram.tile([2, S], F32, name=f"rsd_{e}")
        nc.sync.dma_start(out=rsd[0:1, :], in_=rs_a)
        nc.sync.dma_start(out=rsd[1:2, :], in_=rs_b)
        recB = epl.tile([64, S], F32, name=f"recB_{e}")
        nc.sync.dma_start(out=recB[0:32, :],
                          in_=bass.AP(tensor=rsd.tensor, offset=0, ap=[[0, 32], [1, S]]))
        nc.sync.dma_start(out=recB[32:64, :],
                          in_=bass.AP(tensor=rsd.tensor, offset=S, ap=[[0, 32], [1, S]]))
        hn = epl.tile([64, S], F32, name=f"hn_{e}")
        nc.vector.tensor_tensor(out=hn, in0=h12, in1=recB, op=ALU.mult)
        nc.scalar.activation(hn, hn, AF.Identity, bias=vbT[e])
        if dbg and e == "enc1":
            nc.sync.dma_start(out=dbg["h12"].ap(), in_=h12)
            nc.sync.dma_start(out=dbg["rs"].ap()[0:1, :], in_=rs_a)
            nc.sync.dma_start(out=dbg["rs"].ap()[32:33, :], in_=rs_b)
            nc.sync.dma_start(out=dbg["hn"].ap(), in_=hn)

        with tc.tile_pool(name=f"ep_{e}", bufs=2, space="PSUM") as ps_ep, \
             tc.tile_pool(name=f"ept_{e}", bufs=2, space="PSUM") as ps_et:
            ln1 = epl.tile([128, IT, D], F32, name=f"ln1_{e}")
            for it in range(IT):
                aps = ps_ep.tile([128, D], F32, name="att_ps", tag="att_ps")
                nc.tensor.matmul(aps, hn[:, it * 128:(it + 1) * 128], ow[e])
                a1 = scr.tile([128, D], F32, name="a1", tag="a1")
                nc.vector.tensor_tensor(out=a1, in0=aps, in1=bc[f"{e}_ob"], op=ALU.add)
                nc.vector.tensor_tensor(out=a1, in0=a1, in1=r_sb[e][:, it, :], op=ALU.add)
                layernorm(ln1[:, it, :], a1, bc[f"{e}_ln_g"], bc[f"{e}_ln_b"])
            ln1T = epl.tile([64, S], F32, name=f"ln1T_{e}")
            for it in range(IT):
                tp = ps_et.tile([64, 128], F32, name="eptp", tag="eptp")
                nc.tensor.transpose(tp, ln1[:, it, :], idf)
                nc.vector.tensor_copy(out=ln1T[:, it * 128:(it + 1) * 128], in_=tp)
            relu1 = epl.tile([128, IT, D], F32, name=f"relu1_{e}")
            for it in range(IT):
                fps = ps_ep.tile([128, D], F32, name="ff1_ps", tag="ff1_ps")
                nc.tensor.matmul(fps, ln1T[:, it * 128:(it + 1) * 128], f1w[e])
                rl = scr.tile([128, D], F32, name="rl", tag="rl")
                nc.vector.tensor_tensor(out=rl, in0=fps, in1=bc[f"{e}_f1b"], op=ALU.add)
                nc.scalar.activation(relu1[:, it, :], rl, AF.Relu)
            if dbg and e == "enc1":
                nc.sync.dma_start(
                    out=dbg["ln1"].ap().rearrange("(it p) d -> p it d", it=IT), in_=ln1)
            rT = epl.tile([64, S], F32, name=f"rT_{e}")
            for it in range(IT):
                tp = ps_et.tile([64, 128], F32, name="eptp2", tag="eptp")
                nc.tensor.transpose(tp, relu1[:, it, :], idf)
                nc.vector.tensor_copy(out=rT[:, it * 128:(it + 1) * 128], in_=tp)
            for it in range(IT):
                fps = ps_ep.tile([128, D], F32, name="ff2_ps", tag="ff2_ps")
                nc.tensor.matmul(fps, rT[:, it * 128:(it + 1) * 128], f2w[e])
                rl = scr.tile([128, D], F32, name="rl2", tag="rl")
                nc.vector.tensor_tensor(out=rl, in0=fps, in1=bc[f"{e}_f2b"], op=ALU.add)
                nc.vector.tensor_tensor(out=rl, in0=rl, in1=ln1[:, it, :], op=ALU.add)
                layernorm(ef[e][:, it, :], rl, bc[f"{e}_ln_g"], bc[f"{e}_ln_b"])
        epc.close()

    # =========================================================
    # PHASE 3: adjT build + spatial spmm + atten combine
    # =========================================================
    p3c = ExitStack()
    p3 = p3c.enter_context(tc.tile_pool(name="p3", bufs=1))
    with tc.tile_pool(name="adjs", bufs=2) as p_as, \
         tc.tile_pool(name="adjtp", bufs=2, space="PSUM") as p_atp, \
         tc.tile_pool(name="esps", bufs=2, space="PSUM") as p_es:
        for jc in range(JC):
            af32 = p_as.tile([128, IT, 512], F32, name="af32", tag="af32")
            nc.sync.dma_start(out=af32,
                              in_=adjd.ap().rearrange("(it p) (jc w) -> p it jc w",
                                                      it=IT, jc=JC)[:, :, jc, :])
            abf = p_as.tile([128, IT, 512], BF16, name="abf", tag="abf")
            nc.gpsimd.tensor_copy(out=abf, in_=af32)
            for jtw in range(4):
                tp = p_atp.tile([128, 512], BF16, name="atp", tag="atp")
                for it in range(IT):
                    nc.tensor.transpose(tp[:, it * 128:(it + 1) * 128],
                                        abf[:, it, jtw * 128:(jtw + 1) * 128], idb)
                nc.vector.tensor_copy(out=adjT[:, jc * 4 + jtw, :], in_=tp)

        spmm = p3.tile([128, IT, 128], F32, name="spmm")
        for it in range(IT):
            eps_ = p_es.tile([128, 128], F32, name="es_ps", tag="es_ps")
            for jt in range(JT):
                nc.tensor.matmul(eps_, adjT[:, jt, it * 128:(it + 1) * 128],
                                 xfull[:, jt, :], start=(jt == 0), stop=(jt == JT - 1))
            nc.scalar.copy(spmm[:, it, :], eps_)

    with tc.tile_pool(name="gwps", bufs=2, space="PSUM") as p_gw, \
         tc.tile_pool(name="gwtp", bufs=2, space="PSUM") as p_gt:
        for mi, sp in enumerate(("sp1", "sp2")):
            smT = p3.tile([64, S], F32, name=f"smT_{sp}")
            for it in range(IT):
                tp = p_gt.tile([64, 128], F32, name="smtp", tag="smtp")
                nc.tensor.transpose(tp, spmm[:, it, mi * 64:mi * 64 + 64], idf)
                nc.vector.tensor_copy(out=smT[:, it * 128:(it + 1) * 128], in_=tp)
            for it in range(IT):
                gps = p_gw.tile([128, D], F32, name="gw_ps", tag="gw_ps")
                nc.tensor.matmul(gps, smT[:, it * 128:(it + 1) * 128], gw[sp])
                nc.scalar.activation(es_m[sp][:, it, :], gps, AF.Relu)

    # atten combine -> e1, e2
    e_comb = {}
    with tc.tile_pool(name="atps", bufs=2, space="PSUM") as p_at:
        for mi, (sp, e, a) in enumerate((("sp1", "enc1", "att1"), ("sp2", "enc2", "att2"))):
            esT = p3.tile([64, S], F32, name=f"esT_{a}")
            efT = p3.tile([64, S], F32, name=f"efT_{a}")
            for it in range(IT):
                tp = p_at.tile([64, 128], F32, name="attp", tag="attp")
                nc.tensor.transpose(tp, es_m[sp][:, it, :], idf)
                nc.vector.tensor_copy(out=esT[:, it * 128:(it + 1) * 128], in_=tp)
                tp2 = p_at.tile([64, 128], F32, name="attp2", tag="attp")
                nc.tensor.transpose(tp2, ef[e][:, it, :], idf)
                nc.vector.tensor_copy(out=efT[:, it * 128:(it + 1) * 128], in_=tp2)
            ga = [p3.tile([1, S], F32, name=f"ga{j}_{a}") for j in range(2)]
            for xi, xT in enumerate((esT, efT)):
                vps = p_at.tile([64, S], F32, name="v_ps", tag="v_ps")
                nc.tensor.matmul(vps, attw[a], xT)
                vth = scr.tile([64, S], F32, name="vth", tag="vth")
                nc.scalar.activation(vth, vps, AF.Exp, scale=2.0)
                nc.vector.tensor_scalar(out=vth, in0=vth, scalar1=1.0, scalar2=None,
                                        op0=ALU.add)
                nc.vector.reciprocal(out=vth, in_=vth)
                nc.vector.tensor_scalar(out=vth, in0=vth, scalar1=-2.0, scalar2=1.0,
                                        op0=ALU.mult, op1=ALU.add)
                gps = p_at.tile([1, S], F32, name="g_ps", tag="g_ps")
                nc.tensor.matmul(gps, attu[a], vth)
                nc.scalar.activation(ga[xi], gps, AF.Exp)
            den = scr.tile([1, S], F32, name="den", tag="den")
            nc.vector.tensor_tensor(out=den, in0=ga[0], in1=ga[1], op=ALU.add)
            nc.vector.reciprocal(out=den, in_=den)
            nc.vector.tensor_tensor(out=ga[0], in0=ga[0], in1=den, op=ALU.mult)
            nc.vector.tensor_tensor(out=ga[1], in0=ga[1], in1=den, op=ALU.mult)
            alpha_d = dram.tile([2, S], F32, name=f"alpha_d_{a}")
            nc.sync.dma_start(out=alpha_d[0:1, :], in_=ga[0])
            nc.sync.dma_start(out=alpha_d[1:2, :], in_=ga[1])
            alphaT = p3.tile([128, IT, 2], F32, name=f"alphaT_{a}")
            for xi2 in range(2):
                nc.sync.dma_start(out=alphaT[:, :, xi2:xi2 + 1],
                                  in_=alpha_d[xi2:xi2 + 1, :]
                                  .rearrange("o (it p) -> p it o", it=IT))
            et = gt([128, IT, D], F32, f"ecomb_{a}")
            for it in range(IT):
                tmp = scr.tile([128, D], F32, name="etmp", tag="etmp")
                nc.vector.tensor_scalar(out=tmp, in0=es_m[sp][:, it, :],
                                        scalar1=alphaT[:, it, 0:1], scalar2=None,
                                        op0=ALU.mult)
                nc.vector.scalar_tensor_tensor(out=et[:, it, :], in0=ef[e][:, it, :],
                                               scalar=alphaT[:, it, 1:2], in1=tmp,
                                               op0=ALU.mult, op1=ALU.add)
            e_comb[mi + 1] = et
    p3c.close()
    enc_ctx.close()
    e1sb, e2sb = e_comb[1], e_comb[2]

    # =========================================================
    # PHASE 4: cross/caus prep, hypergraph partials, AG-B
    # =========================================================
    xc_ctx = ExitStack()
    xc = xc_ctx.enter_context(tc.tile_pool(name="xc", bufs=1))
    with tc.tile_pool(name="p4ps", bufs=2, space="PSUM") as p_p4, \
         tc.tile_pool(name="p4tp", bufs=2, space="PSUM") as p_p4t, \
         tc.tile_pool(name="p4sb", bufs=1) as p4sb:
        catT = xc.tile([128, S], F32, name="catT")
        for it in range(IT):
            tp = p_p4t.tile([64, 128], F32, name="p4tp1", tag="p4tp")
            nc.tensor.transpose(tp, e1sb[:, it, :], idf)
            nc.vector.tensor_copy(out=catT[0:64, it * 128:(it + 1) * 128], in_=tp)
            tp2 = p_p4t.tile([64, 128], F32, name="p4tp2", tag="p4tp")
            nc.tensor.transpose(tp2, e2sb[:, it, :], idf)
            nc.vector.tensor_copy(out=catT[64:128, it * 128:(it + 1) * 128], in_=tp2)

        QTstack2 = xc.tile([128, S], F32, name="QTstack2")
        qps = p_p4.tile([64, S], F32, name="qc_ps", tag="qc_ps")
        nc.tensor.matmul(qps, crossw["qw"], catT[0:64, :])
        nc.scalar.copy(QTstack2[0:64, :], qps)
        qps2 = p_p4.tile([64, S], F32, name="qs_ps", tag="qc_ps")
        nc.tensor.matmul(qps2, causw["qw"], catT[0:64, :])
        nc.scalar.activation(QTstack2[64:128, :], qps2, AF.Identity, bias=qbcT)

        # dv_is = 1/(sqrt(dv)+1e-6)
        nc.vector.tensor_reduce(out=dvis, in_=H_sb, axis=AX.X, op=ALU.add)
        nc.scalar.activation(dvis, dvis, AF.Ln)
        nc.scalar.activation(dvis, dvis, AF.Exp, scale=0.5)
        nc.vector.tensor_scalar(out=dvis, in0=dvis, scalar1=1e-6, scalar2=None,
                                op0=ALU.add)
        nc.vector.reciprocal(out=dvis, in_=dvis)

        Zt = p4sb.tile([128, IT, D], F32, name="Zt")
        for it in range(IT):
            xps = p_p4.tile([128, D], F32, name="x_ps", tag="x_ps")
            nc.tensor.matmul(xps, catT[:, it * 128:(it + 1) * 128], hgsb)
            nc.vector.tensor_scalar(out=Zt[:, it, :], in0=xps,
                                    scalar1=dvis[:, it, :], scalar2=None, op0=ALU.mult)
        tpart = p4sb.tile([128, IT, D], F32, name="tpart")
        for net in range(IT):
            tps = p_p4.tile([128, D], F32, name="t_ps", tag="t_ps")
            for it in range(IT):
                nc.tensor.matmul(tps, H_sb[:, it, net * 128:(net + 1) * 128],
                                 Zt[:, it, :], start=(it == 0), stop=(it == IT - 1))
            nc.vector.tensor_copy(out=tpart[:, net, :], in_=tps)

        agB_in = dram.tile([2 * S, D], F32, name="agB_in")
        agB_out = dram.tile([2 * S * NCORES, D], F32, name="agB_out",
                            addr_space="Shared")
        nc.sync.dma_start(out=agB_in[0:S, :].rearrange("(it p) d -> p it d", it=IT),
                          in_=e2sb)
        nc.sync.dma_start(out=agB_in[S:2 * S, :].rearrange("(net p) d -> p net d", net=IT),
                          in_=tpart)
        nc.gpsimd.collective_compute(
            "AllGather", ALU.bypass, replica_groups=RG,
            ins=[agB_in.opt()], outs=[agB_out.opt()])

    # =========================================================
    # PHASE 5: cross/caus K,V + attention, hypergraph finish
    # =========================================================
    # hypergraph partial sum first (frees tacc early)
    t_sum = xc.tile([128, IT, D], F32, name="t_sum")
    with tc.tile_pool(name="tac", bufs=1) as tac:
        tacc = tac.tile([128, IT, NCORES, D], F32, name="tacc")
        for r in range(NCORES):
            nc.sync.dma_start(out=tacc[:, :, r, :],
                              in_=agB_out[2 * S * r + S: 2 * S * r + 2 * S, :]
                              .rearrange("(net p) d -> p net d", net=IT))
        nc.vector.tensor_tensor(out=tacc[:, :, 0:4, :], in0=tacc[:, :, 0:4, :],
                                in1=tacc[:, :, 4:8, :], op=ALU.add)
        nc.vector.tensor_tensor(out=tacc[:, :, 0:2, :], in0=tacc[:, :, 0:2, :],
                                in1=tacc[:, :, 2:4, :], op=ALU.add)
        nc.vector.tensor_tensor(out=t_sum, in0=tacc[:, :, 0, :], in1=tacc[:, :, 1, :],
                                op=ALU.add)
    W2 = xc.tile([128, IT, D], F32, name="W2")
    for net in range(IT):
        nc.vector.tensor_scalar(out=W2[:, net, :], in0=t_sum[:, net, :],
                                scalar1=deinvT[:, net, :], scalar2=None, op0=ALU.mult)
    with tc.tile_pool(name="hyps", bufs=2, space="PSUM") as p_hy, \
         tc.tile_pool(name="hytp", bufs=2, space="PSUM") as p_hyt, \
         tc.tile_pool(name="htpool", bufs=1) as htpool:
        HT = htpool.tile([128, IT, S], F32, name="HT")
        for it in range(IT):
            for net in range(IT):
                tp = p_hyt.tile([128, 128], F32, name="hytp1", tag="hytp")
                nc.tensor.transpose(tp, H_sb[:, it, net * 128:(net + 1) * 128], idf)
                nc.vector.tensor_copy(out=HT[:, net, it * 128:(it + 1) * 128], in_=tp)
        for it in range(IT):
            hps = p_hy.tile([128, D], F32, name="hy_ps", tag="hy_ps")
            for net in range(IT):
                nc.tensor.matmul(hps, HT[:, net, it * 128:(it + 1) * 128], W2[:, net, :],
                                 start=(net == 0), stop=(net == IT - 1))
            nc.vector.tensor_scalar(out=hyp[:, it, :], in0=hps,
                                    scalar1=dvis[:, it, :], scalar2=None, op0=ALU.mult)
            nc.scalar.activation(hyp[:, it, :], hyp[:, it, :], AF.Relu)

    with tc.tile_pool(name="p5tp", bufs=2, space="PSUM") as p_p5t, \
         tc.tile_pool(name="p5ps", bufs=2, space="PSUM") as p_p5, \
         tc.tile_pool(name="p5io", bufs=1) as p5io:
        e2full = p5io.tile([128, JT, D], F32, name="e2full")
        for r in range(NCORES):
            nc.sync.dma_start(out=e2full[:, 4 * r:4 * r + 4, :],
                              in_=agB_out[2 * S * r: 2 * S * r + S, :]
                              .rearrange("(it p) d -> p it d", it=IT))
        e2T = xc.tile([64, NCORES, S], F32, name="e2T")
        for b in range(NCORES):
            for jtl in range(IT):
                tp = p_p5t.tile([64, 128], F32, name="p5tp1", tag="p5tp")
                nc.tensor.transpose(tp, e2full[:, b * 4 + jtl, :], idf)
                nc.vector.tensor_copy(out=e2T[:, b, jtl * 128:(jtl + 1) * 128], in_=tp)
        KTstack2 = xc.tile([128, NCORES, S], F32, name="KTstack2")
        for b in range(NCORES):
            kps = p_p5.tile([128, S], F32, name="kk_ps", tag="kk_ps")
            nc.tensor.matmul(kps[0:64, :], crossw["kw"], e2T[:, b, :],
                             start=True, stop=True)
            nc.tensor.matmul(kps[64:128, :], causw["kw"], e2T[:, b, :],
                             start=True, stop=True, tile_position=(0, 64))
            nc.scalar.copy(KTstack2[0:64, b, :], kps[0:64, :])
            nc.scalar.activation(KTstack2[64:128, b, :], kps[64:128, :],
                                 AF.Identity, bias=kbcT)
        vv_c = xc.tile([128, JT, 65], BF16, name="vv_c")
        vv_s = xc.tile([128, JT, 65], BF16, name="vv_s")
        nc.vector.memset(vv_c[:, :, 64:65], 1.0)
        nc.vector.memset(vv_s[:, :, 64:65], 1.0)
        for jt in range(JT):
            b, jtl = jt // 4, jt % 4
            vps = p_p5.tile([128, 128], F32, name="vv_ps", tag="vv_ps")
            nc.tensor.matmul(vps[:, 0:64], e2T[:, b, jtl * 128:(jtl + 1) * 128],
                             crossw["vw"], start=True, stop=True)
            nc.tensor.matmul(vps[:, 64:128], e2T[:, b, jtl * 128:(jtl + 1) * 128],
                             causw["vw"], start=True, stop=True)
            nc.vector.tensor_copy(out=vv_c[:, jt, 0:64], in_=vps[:, 0:64])
            nc.vector.tensor_copy(out=vv_s[:, jt, 0:64], in_=vps[:, 64:128])

    groups5 = [
        dict(row_base=0, K=D, masked=False, hM=65, vpv=lambda jt: vv_c[:, jt, :]),
        dict(row_base=64, K=D, masked=True, hM=65, vpv=lambda jt: vv_s[:, jt, :]),
    ]
    hT5, actx5 = attention_pass("xcs", groups5, cmd, 1.0 / float(np.sqrt(D)),
                                QTstack2, KTstack2)

    hn5 = {}
    for gi, nm in ((0, "cross"), (1, "caus")):
        hsb = xc.tile([64, S], F32, name=f"h5_{nm}")
        rs = xc.tile([1, S], F32, name=f"rs5_{nm}")
        nc.scalar.copy(hsb, hT5[gi][0:64, :])
        nc.scalar.copy(rs, hT5[gi][64:65, :])
        nc.vector.reciprocal(out=rs, in_=rs)
        rsd5 = dram.tile([1, S], F32, name=f"rsd5_{nm}")
        nc.sync.dma_start(out=rsd5, in_=rs)
        recB = xc.tile([64, S], F32, name=f"recB5_{nm}")
        nc.sync.dma_start(out=recB,
                          in_=bass.AP(tensor=rsd5.tensor, offset=0, ap=[[0, 64], [1, S]]))
        hn = xc.tile([64, S], F32, name=f"hn5_{nm}")
        nc.vector.tensor_tensor(out=hn, in0=hsb, in1=recB, op=ALU.mult)
        if nm == "caus":
            nc.scalar.activation(hn, hn, AF.Identity, bias=vbT["caus"])
        hn5[nm] = hn
    actx5.close()
    with tc.tile_pool(name="p5e", bufs=2, space="PSUM") as p_p5e:
        for nm, dst in (("cross", att_nat), ("caus", caus_nat)):
            for it in range(IT):
                tp = p_p5e.tile([128, 64], F32, name="p5etp", tag="p5etp")
                nc.tensor.transpose(tp, hn5[nm][:, it * 128:(it + 1) * 128],
                                    idf[0:64, 0:64])
                nc.vector.tensor_copy(out=dst[:, it, :], in_=tp)

    # final1 = e1 + hyper + caus ; final2 = e2 + att
    nc.vector.tensor_tensor(out=fin1, in0=e1sb, in1=hyp, op=ALU.add)
    nc.vector.tensor_tensor(out=fin1, in0=fin1, in1=caus_nat, op=ALU.add)
    nc.vector.tensor_tensor(out=fin2, in0=e2sb, in1=att_nat, op=ALU.add)
    xc_ctx.close()
    hctx.close()

    if dbg:
        for nm, t in (("ef1", ef["enc1"]), ("ef2", ef["enc2"]),
                      ("es1", es_m["sp1"]), ("es2", es_m["sp2"]),
                      ("e1", e1sb), ("e2", e2sb), ("att", att_nat),
                      ("caus", caus_nat), ("hyp", hyp),
                      ("fin1", fin1), ("fin2", fin2)):
            nc.sync.dma_start(
                out=dbg[nm].ap().rearrange("(it p) d -> p it d", it=IT), in_=t)

    # =========================================================
    # PHASE 6: decoder
    # =========================================================
    dec_ctx = ExitStack()
    decp = dec_ctx.enter_context(tc.tile_pool(name="decp", bufs=1))
    with tc.tile_pool(name="yps", bufs=2, space="PSUM") as p_y, \
         tc.tile_pool(name="ytp", bufs=2, space="PSUM") as p_yt, \
         tc.tile_pool(name="ysbp", bufs=1) as ysbp:
        Ysb = ysbp.tile([128, IT, DIN1 + DIN2], BF16, name="Ysb")
        fin1T = ysbp.tile([64, S], F32, name="fin1T")
        fin2T = ysbp.tile([64, S], F32, name="fin2T")
        for it in range(IT):
            tp = p_yt.tile([64, 128], F32, name="ytp1", tag="ytp")
            nc.tensor.transpose(tp, fin1[:, it, :], idf)
            nc.vector.tensor_copy(out=fin1T[:, it * 128:(it + 1) * 128], in_=tp)
            tp2 = p_yt.tile([64, 128], F32, name="ytp2", tag="ytp")
            nc.tensor.transpose(tp2, fin2[:, it, :], idf)
            nc.vector.tensor_copy(out=fin2T[:, it * 128:(it + 1) * 128], in_=tp2)
        for it in range(IT):
            for cc, c0, cw_ in ((0, 0, 512), (1, 512, 512), (2, 1024, 256)):
                yp = p_y.tile([128, 512], F32, name="y_ps", tag="y_ps")
                if cc < 2:
                    nc.tensor.matmul(yp[:, 0:cw_], fin1T[:, it * 128:(it + 1) * 128],
                                     dec1sb[:, c0:c0 + cw_])
                else:
                    nc.tensor.matmul(yp[:, 0:cw_], fin2T[:, it * 128:(it + 1) * 128],
                                     dec2sb)
                nc.vector.tensor_copy(out=Ysb[:, it, c0:c0 + cw_], in_=yp[:, 0:cw_])

        agC_in = dram.tile([S, DIN1 + DIN2], BF16, name="agC_in")
        agC_out = dram.tile([N, DIN1 + DIN2], BF16, name="agC_out",
                            addr_space="Shared")
        nc.sync.dma_start(out=agC_in.rearrange("(it p) c -> p it c", it=IT), in_=Ysb)
        nc.gpsimd.collective_compute(
            "AllGather", ALU.bypass, replica_groups=RG,
            ins=[agC_in.opt()], outs=[agC_out.opt()])

    Yf = decp.tile([128, JT, DIN1 + DIN2], BF16, name="Yf")
    for r in range(NCORES):
        nc.sync.dma_start(out=Yf[:, 4 * r:4 * r + 4, :],
                          in_=agC_out[S * r: S * (r + 1), :]
                          .rearrange("(it p) c -> p it c", it=IT))

    with tc.tile_pool(name="decps", bufs=2, space="PSUM") as p_dec, \
         tc.tile_pool(name="outp", bufs=2) as outp:
        for it in range(IT):
            dp = p_dec.tile([128, DIN1 + DIN2], F32, name="dec_ps", tag="dec_ps")
            for jt in range(JT):
                st = (jt == 0)
                sp_ = (jt == JT - 1)
                lhs = adjT[:, jt, it * 128:(it + 1) * 128]
                nc.tensor.matmul(dp[:, 0:512], lhs, Yf[:, jt, 0:512], start=st, stop=sp_)
                nc.tensor.matmul(dp[:, 512:1024], lhs, Yf[:, jt, 512:1024], start=st, stop=sp_)
                nc.tensor.matmul(dp[:, 1024:1280], lhs, Yf[:, jt, 1024:1280], start=st, stop=sp_)
            ot = outp.tile([128, DIN1 + DIN2], F32, name="out_t", tag="out_t")
            nc.scalar.activation(ot, dp, AF.Relu)
            nc.sync.dma_start(
                out=outd.ap().rearrange("(it p) c -> p it c", it=IT)[:, it, :], in_=ot)
    dec_ctx.close()

    es_ctx.close()


# =========================================================
# host entry
# =========================================================
_NC_CACHE = None
LAST_RESULT = None
LAST_DEBUG = None


def _get_nc():
    global _NC_CACHE
    if _NC_CACHE is None:
        _NC_CACHE = build_kernel()
    return _NC_CACHE


def kernel(features_omics1, features_omics2, coordinates, adj_spatial, H,
           adj_feature_omics1, adj_feature_omics2, causal_mask, params):
    nc = _get_nc()
    flat = _flatten_params(params)
    f1 = np.ascontiguousarray(np.asarray(features_omics1, dtype=np.float32))
    f2 = np.ascontiguousarray(np.asarray(features_omics2, dtype=np.float32))
    co = np.ascontiguousarray(np.asarray(coordinates, dtype=np.float32))
    ad = np.ascontiguousarray(np.asarray(adj_spatial, dtype=np.float32))
    Hm = np.ascontiguousarray(np.asarray(H, dtype=np.float32))
    m1 = np.ascontiguousarray(np.asarray(adj_feature_omics1, dtype=np.int32))
    m2 = np.ascontiguousarray(np.asarray(adj_feature_omics2, dtype=np.int32))
    cm = np.ascontiguousarray(np.asarray(causal_mask, dtype=np.int32))

    in_maps = []
    for c in range(NCORES):
        sl = slice(c * S, (c + 1) * S)
        m = {"f1": f1[sl], "f2": f2[sl], "coords": co[sl], "adj": ad[sl],
             "H": Hm[sl], "m1": m1[sl], "m2": m2[sl], "cm": cm[sl]}
        for k, v in flat.items():
            m[f"p_{k}"] = np.ascontiguousarray(v.astype(np.float32).reshape(PARAM_SHAPES[k]))
        in_maps.append(m)

    global LAST_RESULT, LAST_DEBUG
    res = bass_utils.run_bass_kernel_spmd(nc, in_maps, core_ids=list(range(NCORES)))
    LAST_RESULT = res
    if DEBUG:
        LAST_DEBUG = {k[4:]: np.concatenate([res.results[c][k] for c in range(NCORES)], axis=0)
                      for k in res.results[0] if k.startswith("dbg_")}
    return np.concatenate([res.results[c]["out"] for c in range(NCORES)], axis=0)


if __name__ == "__main__":
    nc = build_kernel()
    print("built ok")
